# revision 27
# baseline (speedup 1.0000x reference)
"""ECC / GNN message-passing kernel for 8 Trainium2 NeuronCores (Bass/Tile).

Reference computation (per edge type t of 7):
    h   = x @ W[t] + b[t]                      # [B, N, F] dense projection
    msg = adj[t] * h[:, end_nodes[t], :]       # gather + edge scale
    out[:, start_nodes[t], :] += msg           # scatter-add

This kernel commutes the linear projection past the gather/scatter:
    out = sum_t (S_t @ x) @ W[t] + b[t] (x) deg_t
where S_t is the weighted adjacency (S_t[s,e] = sum adj over edges s<-e) and
deg_t = S_t @ 1. deg_t is precomputed on the host (O(E) scalar work) and the
bias term is applied on-device as a K=7 matmul b^T @ deg folded into the same
PSUM accumulation as the W folds; everything O(E*F) runs on device.

Sharding: destination (start-node) range partitioning, 12500 dests/core.
Per core:
  - x is stored batch-interleaved fp16 ([N, 2F], 512B rows) so a single
    dma_gather descriptor fetches both batches' features for an end node.
  - dma_gather uses int16 indices, so x is split into 5 windows of 20000
    rows; edges are grouped (window, dest-block, type) and padded to
    128-edge chunks (pad edges get weight 0, spread gather rows -- clustering
    them on one row measurably serializes a DMA channel). Slots within each
    chunk are sorted by gather row for HBM locality. Gather calls are kept
    small (~7 chunks) and spread round-robin over all 4 SWDGE queues with a
    deep g-tile pool: on HW the queues genuinely overlap (1 queue is ~3.4x
    slower), and many small in-flight calls keep them all fed.
  - One-hot scatter matrices are built dest-minor (contiguous matmul rhs --
    a d-strided rhs runs ~3x slower on the real PE) in two DVE passes,
    (iota==sl) then *adj. Every operand keeps a packed 2-byte last dim so
    both passes hit the DVE 2x fp16 perf mode: sl/adj are stored as
    duplicated pairs [128, chunk, 2] and broadcast with an innermost
    [stride 1, count 2] AP. Builds cover a whole (superblock x window) chunk
    range in one op pair. (On HW, DVE elementwise time serializes ~1:1 with
    the gather DMA stream, so this pass is sized to its floor; GPSIMD
    offload and per-chunk one-pass variants both measured slower.)
  - Scatter-add runs on the TensorEngine in fp16: per chunk and batch, the
    gathered rows [128e, F] (stationary) multiply the one-hot [128e, 128d]
    (moving), accumulating y_T[f, d] in fp32 PSUM. All 7 types of one block
    share a single [F, 7, 128] PSUM tile (2 banks), drained by ONE
    Activation-engine copy per (block, batch) into the 4-block-wide fp16
    staging tile consumed by the batched fold.
  - The batched fold accumulates sum_t W_t^T @ y_T plus the bias matmul
    b^T[7,F] @ degT[7, 4*128d] into one PSUM bank, which is copied (Act) to
    SBUF fp16 and stored feature-major [B, F, NPAD]; the host transposes and
    widens to fp32.

fp16 data path with fp32 PSUM accumulation; relative error vs the fp32
reference is ~4.6e-4 (gate is 2e-2).
"""
import os
import numpy as np

import concourse.bass as bass
import concourse.mybir as mybir
import concourse.tile as tile
from concourse import bacc
from concourse._compat import get_trn_type
from concourse.bass_utils import run_bass_kernel_spmd

B, N, E, F, T = 2, 100000, 400000, 128, 7
CORES = 8
NPC = N // CORES            # 12500 dests per core
NBLK = (NPC + 127) // 128   # 98 dest blocks per core
NPAD = NBLK * 128
NW = 5                      # gather windows (int16 idx limit is 32768 rows)
WS = N // NW                # 20000
CALLCH = int(os.environ.get("K_CALLCH", "7"))  # chunks per dma_gather call


def _pack_inputs(x, W, b, adj_values, end_nodes, start_nodes):
    """Group edges by (core, window, dest-block, type); build the shared
    chunk structure (max over cores => one SPMD program), per-core gather
    indices, one-hot scalar columns, and the degree vectors."""
    starts = start_nodes.astype(np.int64).ravel()
    ends = end_nodes.astype(np.int64).ravel()
    adj = adj_values.astype(np.float32).ravel()
    types = np.repeat(np.arange(T, dtype=np.int64), E)

    core = starts // NPC
    sl_all = starts % NPC
    blk = sl_all // 128
    sl_blk = sl_all % 128
    win = ends // WS
    idx16 = (ends % WS).astype(np.int16)

    ng_pc = NW * NBLK * T
    gid_local = (win * NBLK + blk) * T + types
    gid = core * ng_pc + gid_local
    counts = np.bincount(gid, minlength=CORES * ng_pc).reshape(CORES, NW, NBLK, T)
    c_cnt = np.ceil(counts.max(axis=0) / 128).astype(np.int64)  # [NW, NBLK, T]

    chunk_off = np.zeros(ng_pc + 1, np.int64)
    np.cumsum(c_cnt.reshape(-1), out=chunk_off[1:])
    total_chunks = int(chunk_off[-1])
    total_slots = total_chunks * 128

    order = np.argsort(gid, kind="stable")
    sorted_key = gid[order]
    newrun = np.empty(len(sorted_key), bool)
    newrun[0] = True
    newrun[1:] = sorted_key[1:] != sorted_key[:-1]
    run_ids = np.cumsum(newrun) - 1
    run_first = np.flatnonzero(newrun)
    rank = np.arange(len(sorted_key)) - run_first[run_ids]
    slot_sorted = chunk_off[gid_local[order]] * 128 + rank

    # Pad slots get weight 0; spread their gather rows across the window so
    # the padding descriptors don't all hammer the same HBM line.
    if os.environ.get("K_PADMODE", "spread") == "same":
        pad_rows = np.zeros(total_slots, np.int16)
    else:
        pad_rows = (np.arange(total_slots, dtype=np.int64) % WS).astype(np.int16)
    idx_flat = np.broadcast_to(pad_rows, (CORES, total_slots)).copy()
    a_flat = np.zeros((CORES, total_slots), np.float16)
    sl_flat = np.zeros((CORES, total_slots), np.float16)
    core_sorted = core[order]
    idx_flat[core_sorted, slot_sorted] = idx16[order]
    a_flat[core_sorted, slot_sorted] = adj[order].astype(np.float16)
    sl_flat[core_sorted, slot_sorted] = sl_blk[order].astype(np.float16)

    if os.environ.get("K_SORT", "1") == "1":
        # Sort slots within each chunk by gather row: descriptors visit HBM
        # in ascending row order within the chunk (slot order inside a chunk
        # is free; one-hot columns carry the permutation).
        ic = idx_flat.reshape(CORES, total_chunks, 128)
        perm = np.argsort(ic, axis=2, kind="stable")
        ic[:] = np.take_along_axis(ic, perm, axis=2)
        af2 = a_flat.reshape(CORES, total_chunks, 128)
        af2[:] = np.take_along_axis(af2, perm, axis=2)
        sf2 = sl_flat.reshape(CORES, total_chunks, 128)
        sf2[:] = np.take_along_axis(sf2, perm, axis=2)

    # dma_gather index layout: idx j of a call -> [j % 16, col0 + j // 16],
    # replicated across the 8 groups of 16 partitions.
    idx_wrap = idx_flat.reshape(CORES, total_slots // 16, 16).transpose(0, 2, 1)
    idx_arr = np.tile(idx_wrap, (1, 8, 1)).copy()
    sl_cols = sl_flat.reshape(CORES, total_chunks, 128).transpose(0, 2, 1)
    a_cols = a_flat.reshape(CORES, total_chunks, 128).transpose(0, 2, 1)
    # Duplicate each chunk-scalar into a pair so the broadcast AP's innermost
    # dim is [stride 1, count 2] (packed 2-byte) => DVE 2x perf mode while the
    # one-hot stays dest-minor (contiguous matmul rhs).
    sl2 = np.repeat(sl_cols[:, :, :, None], 2, axis=3).copy()
    a2 = np.repeat(a_cols[:, :, :, None], 2, axis=3).copy()
    slf = sl_cols.astype(np.float32).copy()
    af = a_cols.astype(np.float32).copy()

    # Per-(type, dest) weighted degrees; bias is applied on-device as
    # b^T[7,F] @ degT[7, d].
    degT = np.zeros((T, N), np.float32)
    for t in range(T):
        degT[t] = np.bincount(start_nodes[t].astype(np.int64),
                              weights=adj_values[t].astype(np.float64),
                              minlength=N).astype(np.float32)
    degT_pc = np.zeros((CORES, T, NPAD), np.float16)
    degT_pc[:, :, :NPC] = degT.reshape(T, CORES, NPC).transpose(1, 0, 2)

    csum = c_cnt.sum(axis=2)  # [NW, NBLK]
    cmax = int(csum.max())
    iota_rows = np.tile(np.arange(128, dtype=np.float16), (128, 1)).reshape(128, 64, 2)

    x_il = np.ascontiguousarray(
        np.concatenate([x[0], x[1]], axis=1).astype(np.float16))  # [N, 2F]

    struct = dict(c_cnt=c_cnt, chunk_off=chunk_off, total_chunks=total_chunks,
                  cmax=cmax)
    per_core = []
    for c in range(CORES):
        per_core.append({
            "x": x_il,
            "Wt": np.ascontiguousarray(W.astype(np.float16)),
            "bt7": np.ascontiguousarray(b.astype(np.float16)),  # [7, F]
            "idx": idx_arr[c],
            "sl": sl2[c],
            "av": a2[c],
            "slf": slf[c],
            "avf": af[c],
            "deg": degT_pc[c],
            "iota": iota_rows,
        })
    return struct, per_core


def build_kernel(c_cnt, chunk_off, total_chunks, cmax):
    DT = mybir.dt.float32
    H = mybir.dt.float16
    total_slots = total_chunks * 128
    NQ = int(os.environ.get("K_NQ", "4"))
    nc = bacc.Bacc(get_trn_type() or "TRN2", target_bir_lowering=False,
                   num_swdge_queues=NQ)

    x_d = nc.dram_tensor("x", [N, B * F], H, kind="ExternalInput")
    W_d = nc.dram_tensor("Wt", [T, F, F], H, kind="ExternalInput")
    b_d = nc.dram_tensor("bt7", [T, F], H, kind="ExternalInput")
    idx_d = nc.dram_tensor("idx", [128, total_slots // 16], mybir.dt.int16,
                           kind="ExternalInput")
    sl_d = nc.dram_tensor("sl", [128, total_chunks, 2], H, kind="ExternalInput")
    a_d = nc.dram_tensor("av", [128, total_chunks, 2], H, kind="ExternalInput")
    slf_d = nc.dram_tensor("slf", [128, total_chunks], DT, kind="ExternalInput")
    avf_d = nc.dram_tensor("avf", [128, total_chunks], DT, kind="ExternalInput")
    deg_d = nc.dram_tensor("deg", [T, NPAD], H, kind="ExternalInput")
    iota_d = nc.dram_tensor("iota", [128, 64, 2], H, kind="ExternalInput")
    # Output is feature-major fp16 [B, F, NPAD]; the host transposes back.
    out_d = nc.dram_tensor("out", [B, F, NPAD], H, kind="ExternalOutput")

    def coff(ww, bb, tt):
        return int(chunk_off[(ww * NBLK + bb) * T + tt])

    coff_raw = coff

    # Per-window gather-call ranges aligned to dest-block boundaries so each
    # block reads exactly one call per window (bounds the live g-tile set).
    calls, call_of_blk = [], []
    for ww in range(NW):
        cs, cmap = [], []
        cur0 = coff(ww, 0, 0)
        for bbx in range(NBLK):
            b1 = coff(ww, bbx + 1, 0) if bbx + 1 < NBLK else (
                coff(ww + 1, 0, 0) if ww + 1 < NW else total_chunks)
            b0 = coff(ww, bbx, 0)
            if b1 - cur0 > CALLCH and b0 > cur0:
                cs.append((cur0, b0))
                cur0 = b0
            cmap.append(len(cs))
        last1 = coff(ww + 1, 0, 0) if ww + 1 < NW else total_chunks
        if last1 > cur0:
            cs.append((cur0, last1))
        cmap = [min(ci, len(cs) - 1) for ci in cmap]
        calls.append(cs)
        call_of_blk.append(cmap)

    G_BUFS = int(os.environ.get("K_GBUFS", "16"))
    MODE = os.environ.get("K_MODE", "full")  # full | gather_only | compute_only
    SB = 4                       # dest blocks folded per batched fold matmul
    NSB = (NBLK + SB - 1) // SB  # superblocks per core
    qrr = [0]

    with tile.TileContext(nc) as tc:
        with (
            tc.tile_pool(name="const", bufs=1) as cpool,
            tc.tile_pool(name="ip", bufs=12) as idxpool,
            tc.tile_pool(name="gp", bufs=G_BUFS) as gpool,
            tc.tile_pool(name="oh", bufs=int(os.environ.get("K_OHBUFS", "14"))) as ohpool,
            tc.tile_pool(name="yt", bufs=int(os.environ.get("K_YTBUFS", "2"))) as ytpool,
            tc.tile_pool(name="ev", bufs=4) as evpool,
            tc.tile_pool(name="dg", bufs=3) as dgpool,
            tc.tile_pool(name="py", bufs=int(os.environ.get("K_PYBUFS", "3")), space="PSUM") as pypool,
            tc.tile_pool(name="po", bufs=int(os.environ.get("K_POBUFS", "2")), space="PSUM") as popool,
        ):
            ACTW = int(os.environ.get("K_ACTW", "0"))
            iota_t = cpool.tile([128, 64, 2], H)
            sl_t = cpool.tile([128, total_chunks, 2], H)
            a_t = cpool.tile([128, total_chunks, 2], H)
            nc.sync.dma_start(out=sl_t[:], in_=sl_d[:])
            nc.sync.dma_start(out=a_t[:], in_=a_d[:])
            if ACTW > 0:
                # fp32 adj scalars for the Act-engine multiply; only the
                # chunks of windows < ACTW (a contiguous prefix) are needed.
                nact = coff_raw(ACTW, 0, 0)
                avf_t = cpool.tile([128, nact], DT)
                nc.sync.dma_start(out=avf_t[:], in_=avf_d[:, :nact])

            nc.sync.dma_start(out=iota_t[:], in_=iota_d[:])
            W_tiles = []
            for t in range(T):
                wt = cpool.tile([F, F], H, tag=f"W{t}")
                nc.sync.dma_start(out=wt[:], in_=W_d[t])
                W_tiles.append(wt)
            b_t = cpool.tile([T, F], H)
            nc.sync.dma_start(out=b_t[:], in_=b_d[:])

            gt = {}

            fake_g = None
            fake_oh = None
            if MODE in ("pe_only", "pe_only_contig", "nogather"):
                fake_g = cpool.tile([128, B * F], H, tag="fakeg")
                nc.sync.dma_start(out=fake_g[:], in_=x_d[0:128, :])
                fake_oh = cpool.tile([128, 128], H, tag="fakeoh")
                nc.sync.dma_start(out=fake_oh[:], in_=x_d[128:256, 0:128])

            def emit_call(ww, ci):
                c0, c1 = calls[ww][ci]
                nch = c1 - c0
                L = nch * 128
                if MODE in ("pe_only", "pe_only_contig", "nogather"):
                    gt[(ww, ci)] = True
                    return
                g = gpool.tile([128, nch, B * F], H, tag="g",
                               name=f"g{ww}_{ci}")
                if MODE == "compute_only":
                    r0 = (c0 * 128) % (N - L)
                    for ch in range(nch):
                        nc.sync.dma_start(
                            out=g[:, ch, :],
                            in_=x_d[r0 + ch * 128:r0 + (ch + 1) * 128, :])
                else:
                    idxp = idxpool.tile([128, nch * 8], mybir.dt.int16,
                                        tag="idxc", name=f"idxc{ww}_{ci}")
                    nc.sync.dma_start(out=idxp[:], in_=idx_d[:, c0 * 8:c1 * 8])
                    nc.gpsimd.dma_gather(
                        g[:],
                        x_d[ww * WS:(ww + 1) * WS, :],
                        idxp[:],
                        L, L, B * F,
                        queue_num=qrr[0] % NQ,
                        single_packet=os.environ.get("K_SP", "0") == "1",
                    )
                qrr[0] += 1
                gt[(ww, ci)] = g

            def chunk_lhsT(ww, bb_, cglob, bt):
                if MODE in ("pe_only", "pe_only_contig", "nogather"):
                    return fake_g[:, bt * F:(bt + 1) * F]
                ci = call_of_blk[ww][bb_]
                c0 = calls[ww][ci][0]
                return gt[(ww, ci)][:, cglob - c0, bt * F:(bt + 1) * F]

            # One-hot tiles are built transposed per (block, window): all the
            # chunks of one block within one window are contiguous in chunk
            # space, and the chunk dim is last so every operand of the two
            # broadcast DVE ops has a packed 2-byte last dim (2x perf mode).
            def build_oh(bb0, bb1, ww):
                c0 = coff(ww, bb0, 0)
                c1 = coff(ww, bb1, 0) if bb1 < NBLK else (
                    coff(ww + 1, 0, 0) if ww + 1 < NW else total_chunks)
                nch = c1 - c0
                oh = ohpool.tile([128, nch, 64, 2], H, tag="oh",
                                 name=f"oh{bb0}_{ww}")
                shp = (128, nch, 64, 2)
                nc.vector.tensor_tensor(
                    out=oh[:],
                    in0=iota_t[:].unsqueeze(1).broadcast_to(shp),
                    in1=sl_t[:, c0:c1, :].unsqueeze(2).broadcast_to(shp),
                    op=mybir.AluOpType.is_equal)
                if ww < ACTW:
                    # per-chunk per-partition scale on the Activation engine
                    # (overlaps the gather chain; DVE does not).
                    for k in range(nch):
                        nc.scalar.mul(out=oh[:, k, :, :], in_=oh[:, k, :, :],
                                      mul=avf_t[:, c0 + k:c0 + k + 1])
                else:
                    nc.vector.tensor_tensor(
                        out=oh[:], in0=oh[:],
                        in1=a_t[:, c0:c1, :].unsqueeze(2).broadcast_to(shp),
                        op=mybir.AluOpType.mult)
                return oh, c0

            REPS = int(os.environ.get("K_REPS", "1"))
            for rep_sb in range(REPS * NSB):
                if rep_sb % NSB == 0:
                    gt.clear()
                sb = rep_sb % NSB
                blks = range(sb * SB, min((sb + 1) * SB, NBLK))
                sbw = len(blks)
                # yTcat[f, t, j, d]: per-type transposed partial sums for the
                # sbw blocks of this superblock, consumed by the batched fold.
                ytcat = {bt: ytpool.tile([F, T, sbw, 128], H, tag="ytc",
                                         name=f"ytc{sb}_{bt}")
                         for bt in range(B)}
                ohs = {}
                for bb in blks:
                    if MODE != "dve_only":
                        for ww in range(NW):
                            ci = call_of_blk[ww][bb]
                            if (ww, ci) not in gt:
                                emit_call(ww, ci)
                    if MODE not in ("gather_only", "pe_only", "pe_only_contig") \
                            and bb == blks[0]:
                        for ww in range(NW):
                            ohs[ww] = build_oh(blks[0], blks[-1] + 1, ww)

                    nonempty = [t for t in range(T)
                                if sum(c_cnt[w2, bb, t] for w2 in range(NW)) > 0]
                    if MODE in ("gather_only", "dve_only", "gather_dve"):
                        continue
                    # All 7 per-type accumulators of this block share one
                    # 2-bank PSUM tile, drained by one copy per batch.
                    full7 = len(nonempty) == T
                    pys = {bt: pypool.tile([F, T, 128], DT, tag="py",
                                           name=f"py{bb}_{bt}")
                           for bt in range(B)}
                    for ti, t in enumerate(nonempty):
                        tot = sum(int(c_cnt[w2, bb, t]) for w2 in range(NW))
                        done = 0
                        for ww in range(NW):
                            for k in range(int(c_cnt[ww, bb, t])):
                                col = coff(ww, bb, t) + k
                                if MODE == "pe_only":
                                    oh_ap = iota_t[:, :, 0:1].broadcast_to(
                                        (128, 128, 1))[:, :, 0]
                                elif MODE == "pe_only_contig":
                                    oh_ap = fake_oh[:]
                                else:
                                    oh_t, c0 = ohs[ww]
                                    oh_ap = oh_t[:, col - c0, :, :]
                                for bt in range(B):
                                    nc.tensor.matmul(
                                        pys[bt][:, t, :],
                                        lhsT=chunk_lhsT(ww, bb, col, bt),
                                        rhs=oh_ap,
                                        start=(done == 0),
                                        stop=(done == tot - 1),
                                    )
                                done += 1
                    j = bb - sb * SB
                    if full7:
                        for bt in range(B):
                            nc.scalar.copy(out=ytcat[bt][:, :, j, :],
                                           in_=pys[bt][:])
                    else:
                        for t in range(T):
                            for bt in range(B):
                                dst = ytcat[bt][:, t, j, :]
                                if t in nonempty:
                                    nc.scalar.copy(out=dst, in_=pys[bt][:, t, :])
                                else:
                                    nc.gpsimd.memset(dst, 0.0)

                if MODE in ("gather_only", "dve_only", "gather_dve"):
                    continue
                # batched fold: out^T[f', d] = sum_t W_t^T @ yT_t over 4
                # blocks, plus the bias matmul b^T @ degT (K=7).
                degb = dgpool.tile([T, sbw * 128], H, tag="degb",
                                   name=f"degb{sb}")
                nc.sync.dma_start(
                    out=degb[:],
                    in_=deg_d[:, sb * SB * 128:sb * SB * 128 + sbw * 128])
                for bt in range(B):
                    pout = popool.tile([128, sbw * 128], DT, tag="pout",
                                       name=f"pout{sb}_{bt}")
                    for t in range(T):
                        nc.tensor.matmul(
                            pout[:],
                            lhsT=W_tiles[t][:],
                            rhs=ytcat[bt][:, t, :, :],
                            start=(t == 0),
                            stop=False,
                        )
                    nc.tensor.matmul(
                        pout[:],
                        lhsT=b_t[:],
                        rhs=degb[:],
                        start=False,
                        stop=True,
                    )
                    out_t = evpool.tile([128, sbw * 128], H, tag="out",
                                        name=f"out{sb}_{bt}")
                    nc.scalar.copy(out=out_t[:], in_=pout[:])
                    nc.sync.dma_start(
                        out=out_d[bt, :, sb * SB * 128:sb * SB * 128 + sbw * 128],
                        in_=out_t[:])

    nc.compile()
    return nc


def kernel(x, W, b, adj_values, end_nodes, start_nodes):
    x = np.asarray(x, dtype=np.float32)
    W = np.asarray(W, dtype=np.float32)
    b = np.asarray(b, dtype=np.float32)
    adj_values = np.asarray(adj_values, dtype=np.float32)
    end_nodes = np.asarray(end_nodes)
    start_nodes = np.asarray(start_nodes)

    struct, per_core = _pack_inputs(x, W, b, adj_values, end_nodes, start_nodes)
    nc = build_kernel(struct["c_cnt"], struct["chunk_off"],
                      struct["total_chunks"], struct["cmax"])
    results = run_bass_kernel_spmd(nc, per_core,
                                   core_ids=list(range(CORES))).results
    out = np.empty((B, N, F), np.float32)
    for c in range(CORES):
        # device output is feature-major fp16 [B, F, NPAD]
        out[:, c * NPC:(c + 1) * NPC, :] = \
            results[c]["out"][:, :, :NPC].astype(np.float32).transpose(0, 2, 1)
    return out


# revision 29
# speedup vs baseline: 1.1938x; 1.1938x over previous
"""ECC / GNN message-passing kernel for 8 Trainium2 NeuronCores (Bass/Tile).

Reference computation (per edge type t of 7):
    h   = x @ W[t] + b[t]                      # [B, N, F] dense projection
    msg = adj[t] * h[:, end_nodes[t], :]       # gather + edge scale
    out[:, start_nodes[t], :] += msg           # scatter-add

This kernel commutes the linear projection past the gather/scatter:
    out = sum_t (S_t @ x) @ W[t] + b[t] (x) deg_t
where S_t is the weighted adjacency (S_t[s,e] = sum adj over edges s<-e) and
deg_t = S_t @ 1. deg_t is precomputed on the host (O(E) scalar work) and the
bias term is applied on-device as a K=7 matmul b^T @ deg folded into the same
PSUM accumulation as the W folds; everything O(E*F) runs on device.

Sharding: destination (start-node) range partitioning, 12500 dests/core.
Per core:
  - x is stored batch-interleaved fp16 ([N, 2F], 512B rows) so a single
    dma_gather descriptor fetches both batches' features for an end node.
  - dma_gather uses int16 indices, so x is split into 5 windows of 20000
    rows; edges are grouped (window, dest-block, type) and padded to
    128-edge chunks (pad edges get weight 0, spread gather rows -- clustering
    them on one row measurably serializes a DMA channel). Slots within each
    chunk are sorted by gather row for HBM locality. Gather calls are kept
    small (~7 chunks) and spread round-robin over all 4 SWDGE queues with a
    deep g-tile pool: on HW the queues genuinely overlap (1 queue is ~3.4x
    slower), and many small in-flight calls keep them all fed.
  - One-hot scatter matrices are built dest-minor (contiguous matmul rhs --
    a d-strided rhs runs ~3x slower on the real PE) in two DVE passes,
    (iota==sl) then *adj. Every operand keeps a packed 2-byte last dim so
    both passes hit the DVE 2x fp16 perf mode: sl/adj are stored as
    duplicated pairs [128, chunk, 2] and broadcast with an innermost
    [stride 1, count 2] AP. Builds cover a whole (superblock x window) chunk
    range in one op pair. (On HW, DVE elementwise time serializes ~1:1 with
    the gather DMA stream, so this pass is sized to its floor; GPSIMD
    offload and per-chunk one-pass variants both measured slower.)
  - Scatter-add runs on the TensorEngine in fp16: per chunk and batch, the
    gathered rows [128e, F] (stationary) multiply the one-hot [128e, 128d]
    (moving), accumulating y_T[f, d] in fp32 PSUM. All 7 types of one block
    share a single [F, 7, 128] PSUM tile (2 banks), drained by ONE
    Activation-engine copy per (block, batch) into the 4-block-wide fp16
    staging tile consumed by the batched fold.
  - The batched fold accumulates sum_t W_t^T @ y_T plus the bias matmul
    b^T[7,F] @ degT[7, 4*128d] into one PSUM bank, which is copied (Act) to
    SBUF fp16 and stored feature-major [B, F, NPAD]; the host transposes and
    widens to fp32.

fp16 data path with fp32 PSUM accumulation; relative error vs the fp32
reference is ~4.6e-4 (gate is 2e-2).
"""
import os
import numpy as np

import concourse.bass as bass
import concourse.mybir as mybir
import concourse.tile as tile
from concourse import bacc
from concourse._compat import get_trn_type
from concourse.bass_utils import run_bass_kernel_spmd

B, N, E, F, T = 2, 100000, 400000, 128, 7
CORES = 8
NPC = N // CORES            # 12500 dests per core
NBLK = (NPC + 127) // 128   # 98 dest blocks per core
NPAD = NBLK * 128
NW = 5                      # gather windows (int16 idx limit is 32768 rows)
WS = N // NW                # 20000
CALLCH = int(os.environ.get("K_CALLCH", "7"))  # chunks per dma_gather call


def _pack_inputs(x, W, b, adj_values, end_nodes, start_nodes):
    """Group edges by (core, window, dest-block, type); build the shared
    chunk structure (max over cores => one SPMD program), per-core gather
    indices, one-hot scalar columns, and the degree vectors."""
    starts = start_nodes.astype(np.int64).ravel()
    ends = end_nodes.astype(np.int64).ravel()
    adj = adj_values.astype(np.float32).ravel()
    types = np.repeat(np.arange(T, dtype=np.int64), E)

    core = starts // NPC
    sl_all = starts % NPC
    blk = sl_all // 128
    sl_blk = sl_all % 128
    win = ends // WS
    idx16 = (ends % WS).astype(np.int16)

    ng_pc = NW * NBLK * T
    gid_local = (win * NBLK + blk) * T + types
    gid = core * ng_pc + gid_local
    counts = np.bincount(gid, minlength=CORES * ng_pc).reshape(CORES, NW, NBLK, T)
    c_cnt = np.ceil(counts.max(axis=0) / 128).astype(np.int64)  # [NW, NBLK, T]

    chunk_off = np.zeros(ng_pc + 1, np.int64)
    np.cumsum(c_cnt.reshape(-1), out=chunk_off[1:])
    total_chunks = int(chunk_off[-1])
    total_slots = total_chunks * 128

    order = np.argsort(gid, kind="stable")
    sorted_key = gid[order]
    newrun = np.empty(len(sorted_key), bool)
    newrun[0] = True
    newrun[1:] = sorted_key[1:] != sorted_key[:-1]
    run_ids = np.cumsum(newrun) - 1
    run_first = np.flatnonzero(newrun)
    rank = np.arange(len(sorted_key)) - run_first[run_ids]
    slot_sorted = chunk_off[gid_local[order]] * 128 + rank

    # Pad slots get weight 0; spread their gather rows across the window so
    # the padding descriptors don't all hammer the same HBM line.
    if os.environ.get("K_PADMODE", "spread") == "same":
        pad_rows = np.zeros(total_slots, np.int16)
    else:
        pad_rows = (np.arange(total_slots, dtype=np.int64) % WS).astype(np.int16)
    idx_flat = np.broadcast_to(pad_rows, (CORES, total_slots)).copy()
    a_flat = np.zeros((CORES, total_slots), np.float16)
    sl_flat = np.zeros((CORES, total_slots), np.float16)
    core_sorted = core[order]
    idx_flat[core_sorted, slot_sorted] = idx16[order]
    a_flat[core_sorted, slot_sorted] = adj[order].astype(np.float16)
    sl_flat[core_sorted, slot_sorted] = sl_blk[order].astype(np.float16)

    if os.environ.get("K_SORT", "1") == "1":
        # Sort slots within each chunk by gather row: descriptors visit HBM
        # in ascending row order within the chunk (slot order inside a chunk
        # is free; one-hot columns carry the permutation).
        ic = idx_flat.reshape(CORES, total_chunks, 128)
        perm = np.argsort(ic, axis=2, kind="stable")
        ic[:] = np.take_along_axis(ic, perm, axis=2)
        af2 = a_flat.reshape(CORES, total_chunks, 128)
        af2[:] = np.take_along_axis(af2, perm, axis=2)
        sf2 = sl_flat.reshape(CORES, total_chunks, 128)
        sf2[:] = np.take_along_axis(sf2, perm, axis=2)

    # dma_gather index layout: idx j of a call -> [j % 16, col0 + j // 16],
    # replicated across the 8 groups of 16 partitions.
    idx_wrap = idx_flat.reshape(CORES, total_slots // 16, 16).transpose(0, 2, 1)
    idx_arr = np.tile(idx_wrap, (1, 8, 1)).copy()
    sl_cols = sl_flat.reshape(CORES, total_chunks, 128).transpose(0, 2, 1)
    a_cols = a_flat.reshape(CORES, total_chunks, 128).transpose(0, 2, 1)
    # Duplicate each chunk-scalar into a pair so the broadcast AP's innermost
    # dim is [stride 1, count 2] (packed 2-byte) => DVE 2x perf mode while the
    # one-hot stays dest-minor (contiguous matmul rhs).
    sl2 = np.repeat(sl_cols[:, :, :, None], 2, axis=3).copy()
    a2 = np.repeat(a_cols[:, :, :, None], 2, axis=3).copy()
    slf = sl_cols.astype(np.float32).copy()
    af = a_cols.astype(np.float32).copy()

    # Per-(type, dest) weighted degrees; bias is applied on-device as
    # b^T[7,F] @ degT[7, d].
    degT = np.zeros((T, N), np.float32)
    for t in range(T):
        degT[t] = np.bincount(start_nodes[t].astype(np.int64),
                              weights=adj_values[t].astype(np.float64),
                              minlength=N).astype(np.float32)
    degT_pc = np.zeros((CORES, T, NPAD), np.float16)
    degT_pc[:, :, :NPC] = degT.reshape(T, CORES, NPC).transpose(1, 0, 2)

    csum = c_cnt.sum(axis=2)  # [NW, NBLK]
    cmax = int(csum.max())
    iota_rows = np.tile(np.arange(128, dtype=np.float16), (128, 1)).reshape(128, 64, 2)

    x_il = np.ascontiguousarray(
        np.concatenate([x[0], x[1]], axis=1).astype(np.float16))  # [N, 2F]

    struct = dict(c_cnt=c_cnt, chunk_off=chunk_off, total_chunks=total_chunks,
                  cmax=cmax)
    per_core = []
    for c in range(CORES):
        per_core.append({
            "x": x_il,
            "Wt": np.ascontiguousarray(W.astype(np.float16)),
            "bt7": np.ascontiguousarray(b.astype(np.float16)),  # [7, F]
            "idx": idx_arr[c],
            "sl": sl2[c],
            "av": a2[c],
            "slf": slf[c],
            "avf": af[c],
            "deg": degT_pc[c],
            "iota": iota_rows,
        })
    return struct, per_core


def build_kernel(c_cnt, chunk_off, total_chunks, cmax):
    DT = mybir.dt.float32
    H = mybir.dt.float16
    total_slots = total_chunks * 128
    NQ = int(os.environ.get("K_NQ", "4"))
    nc = bacc.Bacc(get_trn_type() or "TRN2", target_bir_lowering=False,
                   num_swdge_queues=NQ)

    x_d = nc.dram_tensor("x", [N, B * F], H, kind="ExternalInput")
    W_d = nc.dram_tensor("Wt", [T, F, F], H, kind="ExternalInput")
    b_d = nc.dram_tensor("bt7", [T, F], H, kind="ExternalInput")
    idx_d = nc.dram_tensor("idx", [128, total_slots // 16], mybir.dt.int16,
                           kind="ExternalInput")
    sl_d = nc.dram_tensor("sl", [128, total_chunks, 2], H, kind="ExternalInput")
    a_d = nc.dram_tensor("av", [128, total_chunks, 2], H, kind="ExternalInput")
    slf_d = nc.dram_tensor("slf", [128, total_chunks], DT, kind="ExternalInput")
    avf_d = nc.dram_tensor("avf", [128, total_chunks], DT, kind="ExternalInput")
    deg_d = nc.dram_tensor("deg", [T, NPAD], H, kind="ExternalInput")
    iota_d = nc.dram_tensor("iota", [128, 64, 2], H, kind="ExternalInput")
    # Output is feature-major fp16 [B, F, NPAD]; the host transposes back.
    out_d = nc.dram_tensor("out", [B, F, NPAD], H, kind="ExternalOutput")

    def coff(ww, bb, tt):
        return int(chunk_off[(ww * NBLK + bb) * T + tt])

    coff_raw = coff

    # Per-window gather-call ranges aligned to dest-block boundaries so each
    # block reads exactly one call per window (bounds the live g-tile set).
    calls, call_of_blk = [], []
    for ww in range(NW):
        cs, cmap = [], []
        cur0 = coff(ww, 0, 0)
        for bbx in range(NBLK):
            b1 = coff(ww, bbx + 1, 0) if bbx + 1 < NBLK else (
                coff(ww + 1, 0, 0) if ww + 1 < NW else total_chunks)
            b0 = coff(ww, bbx, 0)
            if b1 - cur0 > CALLCH and b0 > cur0:
                cs.append((cur0, b0))
                cur0 = b0
            cmap.append(len(cs))
        last1 = coff(ww + 1, 0, 0) if ww + 1 < NW else total_chunks
        if last1 > cur0:
            cs.append((cur0, last1))
        cmap = [min(ci, len(cs) - 1) for ci in cmap]
        calls.append(cs)
        call_of_blk.append(cmap)

    G_BUFS = int(os.environ.get("K_GBUFS", "16"))
    MODE = os.environ.get("K_MODE", "full")  # full | gather_only | compute_only
    SB = 4                       # dest blocks folded per batched fold matmul
    NSB = (NBLK + SB - 1) // SB  # superblocks per core
    qrr = [0]

    with tile.TileContext(nc) as tc:
        with (
            tc.tile_pool(name="const", bufs=1) as cpool,
            tc.tile_pool(name="ip", bufs=int(os.environ.get("K_IPBUFS", "12"))) as idxpool,
            tc.tile_pool(name="gp", bufs=G_BUFS) as gpool,
            tc.tile_pool(name="oh", bufs=int(os.environ.get("K_OHBUFS", "14"))) as ohpool,
            tc.tile_pool(name="yt", bufs=int(os.environ.get("K_YTBUFS", "2"))) as ytpool,
            tc.tile_pool(name="ev", bufs=4) as evpool,
            tc.tile_pool(name="dg", bufs=3) as dgpool,
            tc.tile_pool(name="py", bufs=int(os.environ.get("K_PYBUFS", "3")), space="PSUM") as pypool,
            tc.tile_pool(name="po", bufs=int(os.environ.get("K_POBUFS", "2")), space="PSUM") as popool,
        ):
            ACTW = int(os.environ.get("K_ACTW", "0"))
            iota_t = cpool.tile([128, 64, 2], H)
            sl_t = cpool.tile([128, total_chunks, 2], H)
            a_t = cpool.tile([128, total_chunks, 2], H)
            nc.sync.dma_start(out=sl_t[:], in_=sl_d[:])
            nc.sync.dma_start(out=a_t[:], in_=a_d[:])
            if ACTW > 0:
                # fp32 adj scalars for the Act-engine multiply; only the
                # chunks of windows < ACTW (a contiguous prefix) are needed.
                nact = coff_raw(ACTW, 0, 0)
                avf_t = cpool.tile([128, nact], DT)
                nc.sync.dma_start(out=avf_t[:], in_=avf_d[:, :nact])

            nc.sync.dma_start(out=iota_t[:], in_=iota_d[:])
            W_tiles = []
            for t in range(T):
                wt = cpool.tile([F, F], H, tag=f"W{t}")
                nc.sync.dma_start(out=wt[:], in_=W_d[t])
                W_tiles.append(wt)
            b_t = cpool.tile([T, F], H)
            nc.sync.dma_start(out=b_t[:], in_=b_d[:])

            gt = {}

            fake_g = None
            fake_oh = None
            if MODE in ("pe_only", "pe_only_contig", "nogather"):
                fake_g = cpool.tile([128, B * F], H, tag="fakeg")
                nc.sync.dma_start(out=fake_g[:], in_=x_d[0:128, :])
                fake_oh = cpool.tile([128, 128], H, tag="fakeoh")
                nc.sync.dma_start(out=fake_oh[:], in_=x_d[128:256, 0:128])

            def emit_call(ww, ci):
                c0, c1 = calls[ww][ci]
                nch = c1 - c0
                L = nch * 128
                if MODE in ("pe_only", "pe_only_contig", "nogather"):
                    gt[(ww, ci)] = True
                    return
                g = gpool.tile([128, nch, B * F], H, tag="g",
                               name=f"g{ww}_{ci}")
                if MODE == "compute_only":
                    r0 = (c0 * 128) % (N - L)
                    for ch in range(nch):
                        nc.sync.dma_start(
                            out=g[:, ch, :],
                            in_=x_d[r0 + ch * 128:r0 + (ch + 1) * 128, :])
                else:
                    idxp = idxpool.tile([128, nch * 8], mybir.dt.int16,
                                        tag="idxc", name=f"idxc{ww}_{ci}")
                    nc.sync.dma_start(out=idxp[:], in_=idx_d[:, c0 * 8:c1 * 8])
                    nc.gpsimd.dma_gather(
                        g[:],
                        x_d[ww * WS:(ww + 1) * WS, :],
                        idxp[:],
                        L, L, B * F,
                        queue_num=qrr[0] % NQ,
                        single_packet=os.environ.get("K_SP", "0") == "1",
                    )
                qrr[0] += 1
                gt[(ww, ci)] = g

            def chunk_lhsT(ww, bb_, cglob, bt):
                if MODE in ("pe_only", "pe_only_contig", "nogather"):
                    return fake_g[:, bt * F:(bt + 1) * F]
                ci = call_of_blk[ww][bb_]
                c0 = calls[ww][ci][0]
                return gt[(ww, ci)][:, cglob - c0, bt * F:(bt + 1) * F]

            # One-hot tiles are built transposed per (block, window): all the
            # chunks of one block within one window are contiguous in chunk
            # space, and the chunk dim is last so every operand of the two
            # broadcast DVE ops has a packed 2-byte last dim (2x perf mode).
            def build_oh(bb0, bb1, ww):
                c0 = coff(ww, bb0, 0)
                c1 = coff(ww, bb1, 0) if bb1 < NBLK else (
                    coff(ww + 1, 0, 0) if ww + 1 < NW else total_chunks)
                nch = c1 - c0
                oh = ohpool.tile([128, nch, 64, 2], H, tag="oh",
                                 name=f"oh{bb0}_{ww}")
                shp = (128, nch, 64, 2)
                nc.vector.tensor_tensor(
                    out=oh[:],
                    in0=iota_t[:].unsqueeze(1).broadcast_to(shp),
                    in1=sl_t[:, c0:c1, :].unsqueeze(2).broadcast_to(shp),
                    op=mybir.AluOpType.is_equal)
                if ww < ACTW:
                    # per-chunk per-partition scale on the Activation engine
                    # (overlaps the gather chain; DVE does not).
                    for k in range(nch):
                        nc.scalar.mul(out=oh[:, k, :, :], in_=oh[:, k, :, :],
                                      mul=avf_t[:, c0 + k:c0 + k + 1])
                else:
                    nc.vector.tensor_tensor(
                        out=oh[:], in0=oh[:],
                        in1=a_t[:, c0:c1, :].unsqueeze(2).broadcast_to(shp),
                        op=mybir.AluOpType.mult)
                return oh, c0

            REPS = int(os.environ.get("K_REPS", "1"))
            for rep_sb in range(REPS * NSB):
                if rep_sb % NSB == 0:
                    gt.clear()
                sb = rep_sb % NSB
                blks = range(sb * SB, min((sb + 1) * SB, NBLK))
                sbw = len(blks)
                # yTcat[f, t, j, d]: per-type transposed partial sums for the
                # sbw blocks of this superblock, consumed by the batched fold.
                ytcat = {bt: ytpool.tile([F, T, sbw, 128], H, tag="ytc",
                                         name=f"ytc{sb}_{bt}")
                         for bt in range(B)}
                ohs = {}
                for bb in blks:
                    if MODE != "dve_only":
                        for ww in range(NW):
                            ci = call_of_blk[ww][bb]
                            if (ww, ci) not in gt:
                                emit_call(ww, ci)
                    if MODE not in ("gather_only", "pe_only", "pe_only_contig") \
                            and bb == blks[0]:
                        for ww in range(NW):
                            ohs[ww] = build_oh(blks[0], blks[-1] + 1, ww)

                    nonempty = [t for t in range(T)
                                if sum(c_cnt[w2, bb, t] for w2 in range(NW)) > 0]
                    if MODE in ("gather_only", "dve_only", "gather_dve"):
                        continue
                    # All 7 per-type accumulators of this block share one
                    # 2-bank PSUM tile, drained by one copy per batch.
                    full7 = len(nonempty) == T
                    pys = {bt: pypool.tile([F, T, 128], DT, tag="py",
                                           name=f"py{bb}_{bt}")
                           for bt in range(B)}
                    for ti, t in enumerate(nonempty):
                        tot = sum(int(c_cnt[w2, bb, t]) for w2 in range(NW))
                        done = 0
                        for ww in range(NW):
                            for k in range(int(c_cnt[ww, bb, t])):
                                col = coff(ww, bb, t) + k
                                if MODE == "pe_only":
                                    oh_ap = iota_t[:, :, 0:1].broadcast_to(
                                        (128, 128, 1))[:, :, 0]
                                elif MODE == "pe_only_contig":
                                    oh_ap = fake_oh[:]
                                else:
                                    oh_t, c0 = ohs[ww]
                                    oh_ap = oh_t[:, col - c0, :, :]
                                for bt in range(B):
                                    nc.tensor.matmul(
                                        pys[bt][:, t, :],
                                        lhsT=chunk_lhsT(ww, bb, col, bt),
                                        rhs=oh_ap,
                                        start=(done == 0),
                                        stop=(done == tot - 1),
                                    )
                                done += 1
                    j = bb - sb * SB
                    if full7:
                        for bt in range(B):
                            nc.scalar.copy(out=ytcat[bt][:, :, j, :],
                                           in_=pys[bt][:])
                    else:
                        for t in range(T):
                            for bt in range(B):
                                dst = ytcat[bt][:, t, j, :]
                                if t in nonempty:
                                    nc.scalar.copy(out=dst, in_=pys[bt][:, t, :])
                                else:
                                    nc.gpsimd.memset(dst, 0.0)

                if MODE in ("gather_only", "dve_only", "gather_dve"):
                    continue
                # batched fold: out^T[f', d] = sum_t W_t^T @ yT_t over 4
                # blocks, plus the bias matmul b^T @ degT (K=7).
                degb = dgpool.tile([T, sbw * 128], H, tag="degb",
                                   name=f"degb{sb}")
                nc.sync.dma_start(
                    out=degb[:],
                    in_=deg_d[:, sb * SB * 128:sb * SB * 128 + sbw * 128])
                for bt in range(B):
                    pout = popool.tile([128, sbw * 128], DT, tag="pout",
                                       name=f"pout{sb}_{bt}")
                    for t in range(T):
                        nc.tensor.matmul(
                            pout[:],
                            lhsT=W_tiles[t][:],
                            rhs=ytcat[bt][:, t, :, :],
                            start=(t == 0),
                            stop=False,
                        )
                    nc.tensor.matmul(
                        pout[:],
                        lhsT=b_t[:],
                        rhs=degb[:],
                        start=False,
                        stop=True,
                    )
                    out_t = evpool.tile([128, sbw * 128], H, tag="out",
                                        name=f"out{sb}_{bt}")
                    nc.scalar.copy(out=out_t[:], in_=pout[:])
                    nc.sync.dma_start(
                        out=out_d[bt, :, sb * SB * 128:sb * SB * 128 + sbw * 128],
                        in_=out_t[:])

    nc.compile()
    return nc


def kernel(x, W, b, adj_values, end_nodes, start_nodes):
    x = np.asarray(x, dtype=np.float32)
    W = np.asarray(W, dtype=np.float32)
    b = np.asarray(b, dtype=np.float32)
    adj_values = np.asarray(adj_values, dtype=np.float32)
    end_nodes = np.asarray(end_nodes)
    start_nodes = np.asarray(start_nodes)

    struct, per_core = _pack_inputs(x, W, b, adj_values, end_nodes, start_nodes)
    nc = build_kernel(struct["c_cnt"], struct["chunk_off"],
                      struct["total_chunks"], struct["cmax"])
    results = run_bass_kernel_spmd(nc, per_core,
                                   core_ids=list(range(CORES))).results
    out = np.empty((B, N, F), np.float32)
    for c in range(CORES):
        # device output is feature-major fp16 [B, F, NPAD]
        out[:, c * NPC:(c + 1) * NPC, :] = \
            results[c]["out"][:, :, :NPC].astype(np.float32).transpose(0, 2, 1)
    return out
